# revision 14
# baseline (speedup 1.0000x reference)
"""Trainium2 Bass kernel for nn_Attention_7962869366891.

Module: y = x + Wo @ attn(LN_q(x) Wq, LN_c(x) Wkv)   with B=4, N=2048, F=1024,
H=16 heads, D=64.

Sharding (8 cores): core c -> (batch b = c//2, head-group g = c%2 of 8 heads).
Each core computes a full [N, F] partial of the output projection for its 8
heads; the host sums the two partials per batch plus the residual skip.

Device-side design (per core), v5:
  - bf16 datapath, fp32 PSUM/stats/normalization.
  - Few big strided DMAs (per-chunk x slabs, whole-weight slabs) -- the sync
    engine was the lead-in bottleneck with per-tile DMAs.
  - Chunk-major LN stats so LN/z and the first projections start early.
  - LN affine folded into weights host-side; per-token -mu*rstd rides as one
    K=2 matmul per accumulation group.
  - Attention blocks of (head, nhalf) x 16 key-tiles: St = k^T q, exp from
    PSUM into bf16 pt, O^T = V'^T P with a ones column for the denominator.
    Block order: all nhalf=0 across pairs, then all nhalf=1, so out-proj
    chunks 0,1 unlock at the halfway point.
  - All projection / out-proj work lives at the END of program order (lowest
    priority): the Tile scheduler pulls it per-instruction exactly when the
    PE idles waiting on exp, keeping the PE dense (HAM stays warm) without
    ever preempting the attention-critical QK stream.
  - o_ps PSUM slots released fast via DVE copies; reciprocal broadcast
    (gpsimd DRAM bounce) runs off-critical-path.
"""

import numpy as np
import ml_dtypes

import concourse.bass as bass
import concourse.bacc as bacc
import concourse.mybir as mybir
import concourse.tile as tile
from concourse.bass_utils import run_bass_kernel_spmd

F32 = mybir.dt.float32
BF16 = mybir.dt.bfloat16
AF = mybir.ActivationFunctionType

B, N, F, H, D = 4, 2048, 1024, 16, 64
HG = 8                # heads per core
E = HG * D            # 512 projection dims per core
NT = N // 128         # 16 token tiles
FTC = F // 128        # 8 feature tiles
ET = E // 128         # 4 e-tiles (head pairs)
NCH = N // 512        # 4 token chunks of 512
EPS = 1e-5

_CACHE = {}


def build_program():
    nc = bacc.Bacc("TRN2", target_bir_lowering=False, debug=False, num_devices=8)

    xT = nc.dram_tensor("xT", [F, N], BF16, kind="ExternalInput").ap()
    wq = nc.dram_tensor("wq", [F, E], BF16, kind="ExternalInput").ap()
    wk = nc.dram_tensor("wk", [F, E], BF16, kind="ExternalInput").ap()
    wv = nc.dram_tensor("wv", [F, E], BF16, kind="ExternalInput").ap()
    corr = nc.dram_tensor("corr", [2, 3 * E], BF16, kind="ExternalInput").ap()
    wo = nc.dram_tensor("wo", [E, F], BF16, kind="ExternalInput").ap()
    onesd = nc.dram_tensor("onesd", [128, 512], BF16, kind="ExternalInput").ap()
    out = nc.dram_tensor("out", [N, F], BF16, kind="ExternalOutput").ap()
    scr = nc.dram_tensor("scr", [HG * NCH, 512], F32).ap()

    with tile.TileContext(nc) as tc:
        _emit(nc, tc, xT, wq, wk, wv, corr, wo, onesd, out, scr)
    nc.compile()
    return nc


def _emit(nc, tc, xT, wq, wk, wv, corr, wo, onesd, out, scr):
    from contextlib import ExitStack
    pers = ExitStack()
    with pers:
        # ---------------- persistent constants ----------------
        single = pers.enter_context(tc.tile_pool(name="single", bufs=1))
        ones128 = single.tile([128, 128], BF16)
        nc.sync.dma_start(out=ones128, in_=onesd[:, 0:128])
        zero_c = single.tile([128, 1], F32)
        nc.vector.memset(zero_c, 0.0)
        eps_c = single.tile([128, 1], F32)
        nc.vector.memset(eps_c, EPS)
        aug = [single.tile([2, 512], BF16, name=f"aug_{c}", tag=f"aug_{c}")
               for c in range(NCH)]
        for c in range(NCH):
            nc.gpsimd.dma_start(out=aug[c][1:2, :], in_=onesd[0:1, :])
        corr2 = single.tile([2, 3 * E], BF16)  # row0 = -rowsum(W'), row1 = bias
        nc.sync.dma_start(out=corr2, in_=corr)

        # ---------------- x: one big strided DMA per chunk ----------------
        xpool = pers.enter_context(tc.tile_pool(name="x", bufs=1))
        xT_r = xT.rearrange("(ft p) n -> p ft n", p=128)
        xc, xc_r = [], []
        for c in range(NCH):
            t = xpool.tile([128, FTC * 512], BF16, name=f"x_{c}", tag=f"x_{c}")
            tr = t.rearrange("p (ft n) -> p ft n", n=512)
            nc.sync.dma_start(out=tr, in_=xT_r[:, :, c * 512:(c + 1) * 512])
            xc.append(t)
            xc_r.append(tr)

        def xts(ft, c):
            return xc_r[c][:, ft, :]

        # ---------------- weights: one slab DMA each ----------------
        wpool = pers.enter_context(tc.tile_pool(name="w", bufs=1))
        wsl = {}
        for wdram, nm in ((wq, "wq"), (wk, "wk"), (wv, "wv")):
            t = wpool.tile([128, FTC * E], BF16, name=f"{nm}t", tag=f"{nm}t")
            tr = t.rearrange("p (ft e) -> p ft e", e=E)
            nc.sync.dma_start(out=tr,
                              in_=wdram.rearrange("(ft p) e -> p ft e", p=128))
            wsl[nm] = tr
        wot = wpool.tile([128, ET * F], BF16, name="wot", tag="wot")
        wot_r = wot.rearrange("p (et f) -> p et f", f=F)
        nc.sync.dma_start(out=wot_r,
                          in_=wo.rearrange("(et p) f -> p et f", p=128))

        # ---------------- LN stats (chunk-major) ----------------
        rp = pers.enter_context(tc.tile_pool(name="rp", bufs=1))
        rb = [rp.tile([128, 512], F32, name=f"rb_{c}", tag=f"rb_{c}")
              for c in range(NCH)]
        with tc.tile_pool(name="pstats", bufs=1, space="PSUM") as pstats, \
             tc.tile_pool(name="xsq", bufs=1) as xsqp, \
             tc.tile_pool(name="statf", bufs=1) as statf:
            s1l, s2l, xsl = [], [], []
            for c in range(NCH):      # all x^2 on DVE up front
                xsl.append([xsqp.tile([128, 512], BF16, tag=f"xs{c}_{ft}",
                                      name=f"xs{c}_{ft}") for ft in range(FTC)])
                for ft in range(FTC):
                    nc.vector.tensor_mul(xsl[c][ft], xts(ft, c), xts(ft, c))
            for c in range(NCH):      # s1 needs no xsq: PE starts on DMA
                s1 = pstats.tile([128, 512], F32, tag=f"s1{c}", name=f"s1_{c}")
                for ft in range(FTC):
                    nc.tensor.matmul(s1, ones128, xts(ft, c),
                                     start=(ft == 0), stop=(ft == FTC - 1))
                s1l.append(s1)
            for c in range(NCH):
                s2 = pstats.tile([128, 512], F32, tag=f"s2{c}", name=f"s2_{c}")
                for ft in range(FTC):
                    nc.tensor.matmul(s2, ones128, xsl[c][ft],
                                     start=(ft == 0), stop=(ft == FTC - 1))
                s2l.append(s2)
            mus, vas = [], []
            for c in range(NCH):
                mu = statf.tile([128, 512], F32, tag=f"mu{c}", name=f"mu_{c}")
                va = statf.tile([128, 512], F32, tag=f"va{c}", name=f"va_{c}")
                m2 = statf.tile([128, 512], F32, tag=f"m2{c}", name=f"m2_{c}")
                nc.vector.tensor_scalar_mul(mu, s1l[c], 1.0 / F)
                nc.vector.tensor_scalar_mul(va, s2l[c], 1.0 / F)
                nc.vector.tensor_mul(m2, mu, mu)
                nc.vector.tensor_sub(va, va, m2)   # var
                mus.append(mu)
                vas.append(va)
            for c in range(NCH):      # batched Ln (one table set)
                nc.scalar.activation(vas[c], vas[c], AF.Ln, bias=eps_c)
            for c in range(NCH):      # batched Exp
                nc.scalar.activation(rb[c], vas[c], AF.Exp,
                                     bias=zero_c, scale=-0.5)
            for c in range(NCH):
                nc.vector.tensor_mul(mus[c], mus[c], rb[c])    # mu*rstd
                nc.vector.tensor_copy(aug[c][0:1, :], mus[c][0:1, :])
                for ft in range(FTC):                          # z = x*rstd
                    nc.vector.tensor_mul(xts(ft, c), xts(ft, c), rb[c])

        # ---------------- result tiles ----------------
        qkpool = pers.enter_context(tc.tile_pool(name="qk", bufs=1, side="right"))
        qt = [[qkpool.tile([128, 512], BF16, name=f"qt_{et}_{c}",
                           tag=f"qt_{et}_{c}") for c in range(NCH)]
              for et in range(ET)]
        kt = [[qkpool.tile([128, 512], BF16, name=f"kt_{et}_{c}",
                           tag=f"kt_{et}_{c}") for c in range(NCH)]
              for et in range(ET)]
        vpool = pers.enter_context(tc.tile_pool(name="vtok", bufs=1, side="right"))
        vt = [vpool.tile([128, HG * (D + 1)], BF16, name=f"vt_{m}",
                         tag=f"vt_{m}") for m in range(NT)]
        vt_r = [t.rearrange("p (h x) -> p h x", x=D + 1) for t in vt]
        opool = pers.enter_context(tc.tile_pool(name="ostk", bufs=1, side="right"))
        ot = [[opool.tile([128, 512], BF16, name=f"ot_{et}_{c}",
                          tag=f"ot_{et}_{c}") for c in range(NCH)]
              for et in range(ET)]
        obp = pers.enter_context(tc.tile_pool(name="obp", bufs=4))

        def qk_group(pool, wnm, wi, et, c, dest):
            crA = corr2[:, wi * E + et * 128: wi * E + (et + 1) * 128]
            ps = pool.tile([128, 512], F32, tag="pp", name=f"pp{wi}_{et}_{c}")
            for ft in range(FTC):
                nc.tensor.matmul(ps, wsl[wnm][:, ft, et * 128:(et + 1) * 128],
                                 xts(ft, c), start=(ft == 0), stop=False)
            nc.tensor.matmul(ps, crA, aug[c], start=False, stop=True)
            nc.vector.tensor_copy(dest[et][c], ps)

        def v_group(pool, m):
            c, js = m // 4, slice((m % 4) * 128, (m % 4 + 1) * 128)
            nc.gpsimd.dma_start(out=vt_r[m][:, :, D:D + 1], in_=onesd[:, 0:HG])
            ps = pool.tile([128, 512], F32, tag="pp", name=f"ppv_{m}")
            for ft in range(FTC):
                nc.tensor.matmul(ps, xts(ft, c)[:, js], wsl["wv"][:, ft, :],
                                 start=(ft == 0), stop=False)
            nc.tensor.matmul(ps, aug[c][:, js], corr2[:, 2 * E:3 * E],
                             start=False, stop=True)
            nc.vector.tensor_copy(vt_r[m][:, :, 0:D],
                                  ps.rearrange("p (h d) -> p h d", d=D))

        def outproj_group(pool, tt, fc):
            c = tt // 4
            js = slice((tt % 4) * 128, (tt % 4 + 1) * 128)
            ts_ = slice(tt * 128, (tt + 1) * 128)
            fs = slice(fc * 512, (fc + 1) * 512)
            ps = pool.tile([128, 512], F32, tag="pp", name=f"pso{tt}_{fc}")
            for et in range(ET):
                nc.tensor.matmul(ps, ot[et][c][:, js], wot_r[:, et, fs],
                                 start=(et == 0), stop=(et == ET - 1))
            ob = obp.tile([128, 512], BF16, tag="ob", name=f"ob{tt}_{fc}")
            nc.vector.tensor_copy(ob, ps)
            nc.sync.dma_start(out=out[ts_, fs], in_=ob)

        qkv = ExitStack()
        with qkv:
            pst = qkv.enter_context(tc.tile_pool(name="pst", bufs=2, space="PSUM"))
            po = qkv.enter_context(tc.tile_pool(name="po", bufs=2, space="PSUM"))
            pwork = qkv.enter_context(tc.tile_pool(name="pwork", bufs=2,
                                                   space="PSUM"))
            ptp = qkv.enter_context(tc.tile_pool(name="ptp", bufs=4))
            dnp = qkv.enter_context(tc.tile_pool(name="dn", bufs=4))
            oup = qkv.enter_context(tc.tile_pool(name="ou", bufs=4))

            filler = []
            fidx = [0]

            def emit_filler(n=1):
                while n > 0 and fidx[0] < len(filler):
                    filler[fidx[0]]()
                    fidx[0] += 1
                    n -= 1

            def attn_block(p, h, nh, emit_at=(2, 5, 8, 11, 14)):
                er = (h % 2) * 64
                o_ps = [po.tile([65, 512], F32, tag="ops",
                                name=f"ops{h}_{nh}_{i}") for i in range(2)]
                for m in range(NT):
                    ms_ = slice(m * 128, (m + 1) * 128)
                    st = pst.tile([128, 1024], F32, tag="st",
                                  name=f"st{h}_{nh}_{m}")
                    kts = kt[p][m // 4][er:er + 64,
                                        (m % 4) * 128:(m % 4 + 1) * 128]
                    for i in range(2):
                        c = 2 * nh + i
                        nc.tensor.matmul(st[:, i * 512:(i + 1) * 512],
                                         kts, qt[p][c][er:er + 64, :],
                                         start=True, stop=True)
                    pt = ptp.tile([128, 1024], BF16, tag="pt",
                                  name=f"pt{h}_{nh}_{m}")
                    nc.scalar.activation(pt, st, AF.Exp, bias=zero_c)
                    for i in range(2):
                        nc.tensor.matmul(o_ps[i], vt_r[m][:, h, :],
                                         pt[:, i * 512:(i + 1) * 512],
                                         start=(m == 0), stop=(m == NT - 1))
                    if m in emit_at:
                        emit_filler(1)
                for i in range(2):
                    c = 2 * nh + i
                    ou = oup.tile([65, 512], F32, tag="ou", name=f"ou{h}_{c}")
                    nc.vector.tensor_copy(ou, o_ps[i])   # frees the PSUM slot
                    den_b = dnp.tile([64, 512], F32, tag="db", name=f"db{h}_{c}")
                    sr = scr[h * NCH + c:h * NCH + c + 1, :]
                    nc.gpsimd.dma_start(out=sr, in_=ou[64:65, :])
                    nc.gpsimd.dma_start(out=den_b,
                                        in_=sr.to_broadcast([64, 512]))
                    rbt = dnp.tile([64, 512], F32, tag="rb", name=f"rbt{h}_{c}")
                    nc.vector.reciprocal_approx_fast(rbt, den_b)
                    nc.vector.tensor_mul(ot[p][c][er:er + 64, :],
                                         ou[0:64, :], rbt)
                emit_filler(1)   # boundary: keep PE fed across block seams

            # -------- schedule: upfront q0/k0 + paced filler inventory.
            # Explicit static interleave: the scheduler's cost-model sim
            # does not model the HAM cold clock, so filler positions are
            # pinned in program order at a measured pace instead.
            for c in range(NCH):
                qk_group(pwork, "wk", 1, 0, c, kt)
            for c in range(NCH):
                qk_group(pwork, "wq", 0, 0, c, qt)
            for m in range(4):
                v_group(pwork, m)

            filler += [lambda m=m: v_group(pwork, m) for m in range(4, NT)]
            for p in (1, 2, 3):
                filler += [lambda c=c, p=p: qk_group(pwork, "wk", 1, p, c, kt)
                           for c in range(NCH)]
                filler += [lambda c=c, p=p: qk_group(pwork, "wq", 0, p, c, qt)
                           for c in range(NCH)]

            attn_block(0, 0, 0, emit_at=tuple(range(12)))   # V rides here
            attn_block(0, 0, 1)
            attn_block(0, 1, 0)
            attn_block(0, 1, 1)
            attn_block(1, 2, 0)
            attn_block(1, 2, 1)
            attn_block(1, 3, 0)
            attn_block(1, 3, 1)
            attn_block(2, 4, 0)
            attn_block(2, 4, 1)
            attn_block(2, 5, 0)
            attn_block(2, 5, 1)
            # pair 3 nh-major: after both nh0 blocks, out-proj chunks 0,1
            # unlock and ride the nh1 blocks.
            attn_block(3, 6, 0)
            attn_block(3, 7, 0)
            filler += [lambda tt=tt, fc=fc: outproj_group(pwork, tt, fc)
                       for tt in range(8) for fc in range(2)]
            attn_block(3, 6, 1, emit_at=(1, 3, 5, 7, 9, 11, 13))
            attn_block(3, 7, 1, emit_at=(1, 3, 5, 7, 9, 11, 13))
            emit_filler(len(filler))

        # -------- tail: out-proj chunks 2,3 (deep PSUM pipeline) --------
        with tc.tile_pool(name="ptail", bufs=4, space="PSUM") as ptail:
            for tt in range(8, NT):
                for fc in range(2):
                    outproj_group(ptail, tt, fc)


def _prep(inputs):
    x = np.asarray(inputs["x"], np.float32)
    Wq = np.asarray(inputs["Wq"], np.float32)
    Wkv = np.asarray(inputs["Wkv"], np.float32)
    Wo = np.asarray(inputs["Wo"], np.float32)
    ln_g = np.asarray(inputs["ln_g"], np.float32)
    ln_b = np.asarray(inputs["ln_b"], np.float32)
    lnc_g = np.asarray(inputs["lnc_g"], np.float32)
    lnc_b = np.asarray(inputs["lnc_b"], np.float32)

    bf = ml_dtypes.bfloat16
    qscale = np.float32(D ** -0.5)
    in_maps = []
    for c in range(8):
        b, g = c // 2, c % 2
        gs = slice(g * E, (g + 1) * E)
        Wq_g = Wq[gs] * ln_g[None, :] * qscale          # [E, F] (scale folded)
        cq = (Wq[gs] @ ln_b) * qscale                   # [E]
        Wk_g = Wkv[gs] * lnc_g[None, :]
        ck = Wkv[gs] @ lnc_b
        Wv_g = Wkv[H * D + g * E:H * D + (g + 1) * E] * lnc_g[None, :]
        cv = Wkv[H * D + g * E:H * D + (g + 1) * E] @ lnc_b
        corr = np.stack([
            np.concatenate([-Wq_g.sum(1), -Wk_g.sum(1), -Wv_g.sum(1)]),
            np.concatenate([cq, ck, cv]),
        ])                                              # [2, 3E]
        in_maps.append({
            "onesd": np.ones((128, 512), bf),
            "xT": np.ascontiguousarray(x[b].T).astype(bf),
            "wq": np.ascontiguousarray(Wq_g.T).astype(bf),
            "wk": np.ascontiguousarray(Wk_g.T).astype(bf),
            "wv": np.ascontiguousarray(Wv_g.T).astype(bf),
            "corr": np.ascontiguousarray(corr).astype(bf),
            "wo": np.ascontiguousarray(Wo[:, gs].T).astype(bf),
        })
    return in_maps


def kernel(**inputs):
    if "nc" not in _CACHE:
        _CACHE["nc"] = build_program()
    nc = _CACHE["nc"]
    in_maps = _prep(inputs)
    res = run_bass_kernel_spmd(nc, in_maps, list(range(8))).results
    x = np.asarray(inputs["x"], np.float32)
    out = np.empty((B, N, F), np.float32)
    for b in range(B):
        out[b] = (res[2 * b]["out"].astype(np.float32)
                  + res[2 * b + 1]["out"].astype(np.float32)
                  + x[b])
    return out


if __name__ == "__main__":
    import reference
    ins = {k: np.asarray(v) for k, v in reference.setup_inputs().items()}
    exp = np.asarray(reference.reference(**ins))
    got = kernel(**ins)
    err = np.abs(got - exp)
    rel = np.linalg.norm(got - exp) / np.linalg.norm(exp)
    print("max abs err:", err.max(), "rel:", rel)


# revision 15
# speedup vs baseline: 1.0165x; 1.0165x over previous
"""Trainium2 Bass kernel for nn_Attention_7962869366891.

Module: y = x + Wo @ attn(LN_q(x) Wq, LN_c(x) Wkv)   with B=4, N=2048, F=1024,
H=16 heads, D=64.

Sharding (8 cores): core c -> (batch b = c//2, head-group g = c%2 of 8 heads).
Each core computes a full [N, F] partial of the output projection for its 8
heads; the host sums the two partials per batch plus the residual skip.

Device-side design (per core), v5:
  - bf16 datapath, fp32 PSUM/stats/normalization.
  - Few big strided DMAs (per-chunk x slabs, whole-weight slabs) -- the sync
    engine was the lead-in bottleneck with per-tile DMAs.
  - Chunk-major LN stats so LN/z and the first projections start early.
  - LN affine folded into weights host-side; per-token -mu*rstd rides as one
    K=2 matmul per accumulation group.
  - Attention blocks of (head, nhalf) x 16 key-tiles: St = k^T q, exp from
    PSUM into bf16 pt, O^T = V'^T P with a ones column for the denominator.
    Block order: all nhalf=0 across pairs, then all nhalf=1, so out-proj
    chunks 0,1 unlock at the halfway point.
  - All projection / out-proj work lives at the END of program order (lowest
    priority): the Tile scheduler pulls it per-instruction exactly when the
    PE idles waiting on exp, keeping the PE dense (HAM stays warm) without
    ever preempting the attention-critical QK stream.
  - o_ps PSUM slots released fast via DVE copies; reciprocal broadcast
    (gpsimd DRAM bounce) runs off-critical-path.
"""

import numpy as np
import ml_dtypes

import concourse.bass as bass
import concourse.bacc as bacc
import concourse.mybir as mybir
import concourse.tile as tile
from concourse.bass_utils import run_bass_kernel_spmd

F32 = mybir.dt.float32
BF16 = mybir.dt.bfloat16
AF = mybir.ActivationFunctionType

B, N, F, H, D = 4, 2048, 1024, 16, 64
HG = 8                # heads per core
E = HG * D            # 512 projection dims per core
NT = N // 128         # 16 token tiles
FTC = F // 128        # 8 feature tiles
ET = E // 128         # 4 e-tiles (head pairs)
NCH = N // 512        # 4 token chunks of 512
EPS = 1e-5

_CACHE = {}


def build_program():
    nc = bacc.Bacc("TRN2", target_bir_lowering=False, debug=False, num_devices=8)

    xT = nc.dram_tensor("xT", [F, N], BF16, kind="ExternalInput").ap()
    wq = nc.dram_tensor("wq", [F, E], BF16, kind="ExternalInput").ap()
    wk = nc.dram_tensor("wk", [F, E], BF16, kind="ExternalInput").ap()
    wv = nc.dram_tensor("wv", [F, E], BF16, kind="ExternalInput").ap()
    corr = nc.dram_tensor("corr", [2, 3 * E], BF16, kind="ExternalInput").ap()
    wo = nc.dram_tensor("wo", [E, F], BF16, kind="ExternalInput").ap()
    onesd = nc.dram_tensor("onesd", [128, 512], BF16, kind="ExternalInput").ap()
    out = nc.dram_tensor("out", [N, F], BF16, kind="ExternalOutput").ap()
    scr = nc.dram_tensor("scr", [HG * NCH, 512], F32).ap()

    with tile.TileContext(nc) as tc:
        _emit(nc, tc, xT, wq, wk, wv, corr, wo, onesd, out, scr)
    nc.compile()
    return nc


def _emit(nc, tc, xT, wq, wk, wv, corr, wo, onesd, out, scr):
    from contextlib import ExitStack
    pers = ExitStack()
    with pers:
        # ---------------- persistent constants ----------------
        single = pers.enter_context(tc.tile_pool(name="single", bufs=1))
        ones128 = single.tile([128, 128], BF16)
        nc.sync.dma_start(out=ones128, in_=onesd[:, 0:128])
        zero_c = single.tile([128, 1], F32)
        nc.vector.memset(zero_c, 0.0)
        eps_c = single.tile([128, 1], F32)
        nc.vector.memset(eps_c, EPS)
        aug = [single.tile([2, 512], BF16, name=f"aug_{c}", tag=f"aug_{c}")
               for c in range(NCH)]
        for c in range(NCH):
            nc.gpsimd.dma_start(out=aug[c][1:2, :], in_=onesd[0:1, :])
        corr2 = single.tile([2, 3 * E], BF16)  # row0 = -rowsum(W'), row1 = bias
        nc.sync.dma_start(out=corr2, in_=corr)

        # ---------------- x: one big strided DMA per chunk ----------------
        xpool = pers.enter_context(tc.tile_pool(name="x", bufs=1))
        xT_r = xT.rearrange("(ft p) n -> p ft n", p=128)
        xc, xc_r = [], []
        for c in range(NCH):
            t = xpool.tile([128, FTC * 512], BF16, name=f"x_{c}", tag=f"x_{c}")
            tr = t.rearrange("p (ft n) -> p ft n", n=512)
            nc.sync.dma_start(out=tr, in_=xT_r[:, :, c * 512:(c + 1) * 512])
            xc.append(t)
            xc_r.append(tr)

        def xts(ft, c):
            return xc_r[c][:, ft, :]

        # ---------------- weights: one slab DMA each ----------------
        wpool = pers.enter_context(tc.tile_pool(name="w", bufs=1))
        wsl = {}
        for wdram, nm in ((wq, "wq"), (wk, "wk"), (wv, "wv")):
            t = wpool.tile([128, FTC * E], BF16, name=f"{nm}t", tag=f"{nm}t")
            tr = t.rearrange("p (ft e) -> p ft e", e=E)
            nc.sync.dma_start(out=tr,
                              in_=wdram.rearrange("(ft p) e -> p ft e", p=128))
            wsl[nm] = tr
        wot = wpool.tile([128, ET * F], BF16, name="wot", tag="wot")
        wot_r = wot.rearrange("p (et f) -> p et f", f=F)
        nc.sync.dma_start(out=wot_r,
                          in_=wo.rearrange("(et p) f -> p et f", p=128))

        # ---------------- LN stats (chunk-major) ----------------
        rp = pers.enter_context(tc.tile_pool(name="rp", bufs=1))
        rb = [rp.tile([128, 512], F32, name=f"rb_{c}", tag=f"rb_{c}")
              for c in range(NCH)]
        with tc.tile_pool(name="pstats", bufs=1, space="PSUM") as pstats, \
             tc.tile_pool(name="xsq", bufs=1) as xsqp, \
             tc.tile_pool(name="statf", bufs=1) as statf:
            s1l, s2l, xsl = [], [], []
            for c in range(NCH):      # all x^2 on DVE up front
                xsl.append([xsqp.tile([128, 512], BF16, tag=f"xs{c}_{ft}",
                                      name=f"xs{c}_{ft}") for ft in range(FTC)])
                for ft in range(FTC):
                    nc.vector.tensor_mul(xsl[c][ft], xts(ft, c), xts(ft, c))
            for c in range(NCH):      # s1 needs no xsq: PE starts on DMA
                s1 = pstats.tile([128, 512], F32, tag=f"s1{c}", name=f"s1_{c}")
                for ft in range(FTC):
                    nc.tensor.matmul(s1, ones128, xts(ft, c),
                                     start=(ft == 0), stop=(ft == FTC - 1))
                s1l.append(s1)
            for c in range(NCH):
                s2 = pstats.tile([128, 512], F32, tag=f"s2{c}", name=f"s2_{c}")
                for ft in range(FTC):
                    nc.tensor.matmul(s2, ones128, xsl[c][ft],
                                     start=(ft == 0), stop=(ft == FTC - 1))
                s2l.append(s2)
            mus, vas = [], []
            for c in range(NCH):
                mu = statf.tile([128, 512], F32, tag=f"mu{c}", name=f"mu_{c}")
                va = statf.tile([128, 512], F32, tag=f"va{c}", name=f"va_{c}")
                m2 = statf.tile([128, 512], F32, tag=f"m2{c}", name=f"m2_{c}")
                nc.vector.tensor_scalar_mul(mu, s1l[c], 1.0 / F)
                nc.vector.tensor_scalar_mul(va, s2l[c], 1.0 / F)
                nc.vector.tensor_mul(m2, mu, mu)
                nc.vector.tensor_sub(va, va, m2)   # var
                mus.append(mu)
                vas.append(va)
            for c in range(NCH):      # batched Ln (one table set)
                nc.scalar.activation(vas[c], vas[c], AF.Ln, bias=eps_c)
            for c in range(NCH):      # batched Exp
                nc.scalar.activation(rb[c], vas[c], AF.Exp,
                                     bias=zero_c, scale=-0.5)
            for c in range(NCH):
                nc.vector.tensor_mul(mus[c], mus[c], rb[c])    # mu*rstd
                nc.vector.tensor_copy(aug[c][0:1, :], mus[c][0:1, :])
                for ft in range(FTC):                          # z = x*rstd
                    nc.vector.tensor_mul(xts(ft, c), xts(ft, c), rb[c])

        # ---------------- result tiles ----------------
        qkpool = pers.enter_context(tc.tile_pool(name="qk", bufs=1, side="right"))
        qt = [qkpool.tile([128, N], BF16, name=f"qt_{et}", tag=f"qt_{et}")
              for et in range(ET)]
        kt = [qkpool.tile([128, N], BF16, name=f"kt_{et}", tag=f"kt_{et}")
              for et in range(ET)]
        vpool = pers.enter_context(tc.tile_pool(name="vtok", bufs=1, side="right"))
        vt = [vpool.tile([128, HG * (D + 1)], BF16, name=f"vt_{m}",
                         tag=f"vt_{m}") for m in range(NT)]
        vt_r = [t.rearrange("p (h x) -> p h x", x=D + 1) for t in vt]
        opool = pers.enter_context(tc.tile_pool(name="ostk", bufs=1, side="right"))
        ot = [[opool.tile([128, 512], BF16, name=f"ot_{et}_{c}",
                          tag=f"ot_{et}_{c}") for c in range(NCH)]
              for et in range(ET)]
        obp = pers.enter_context(tc.tile_pool(name="obp", bufs=4))

        def qk_group(pool, wnm, wi, et, c, dest):
            crA = corr2[:, wi * E + et * 128: wi * E + (et + 1) * 128]
            ps = pool.tile([128, 512], F32, tag="pp", name=f"pp{wi}_{et}_{c}")
            for ft in range(FTC):
                nc.tensor.matmul(ps, wsl[wnm][:, ft, et * 128:(et + 1) * 128],
                                 xts(ft, c), start=(ft == 0), stop=False)
            nc.tensor.matmul(ps, crA, aug[c], start=False, stop=True)
            nc.vector.tensor_copy(dest[et][:, c * 512:(c + 1) * 512], ps)

        def v_group(pool, m):
            c, js = m // 4, slice((m % 4) * 128, (m % 4 + 1) * 128)
            nc.gpsimd.dma_start(out=vt_r[m][:, :, D:D + 1], in_=onesd[:, 0:HG])
            ps = pool.tile([128, 512], F32, tag="pp", name=f"ppv_{m}")
            for ft in range(FTC):
                nc.tensor.matmul(ps, xts(ft, c)[:, js], wsl["wv"][:, ft, :],
                                 start=(ft == 0), stop=False)
            nc.tensor.matmul(ps, aug[c][:, js], corr2[:, 2 * E:3 * E],
                             start=False, stop=True)
            nc.vector.tensor_copy(vt_r[m][:, :, 0:D],
                                  ps.rearrange("p (h d) -> p h d", d=D))

        def outproj_group(pool, tt, fc):
            c = tt // 4
            js = slice((tt % 4) * 128, (tt % 4 + 1) * 128)
            ts_ = slice(tt * 128, (tt + 1) * 128)
            fs = slice(fc * 512, (fc + 1) * 512)
            ps = pool.tile([128, 512], F32, tag="pp", name=f"pso{tt}_{fc}")
            for et in range(ET):
                nc.tensor.matmul(ps, ot[et][c][:, js], wot_r[:, et, fs],
                                 start=(et == 0), stop=(et == ET - 1))
            ob = obp.tile([128, 512], BF16, tag="ob", name=f"ob{tt}_{fc}")
            nc.vector.tensor_copy(ob, ps)
            nc.sync.dma_start(out=out[ts_, fs], in_=ob)

        qkv = ExitStack()
        with qkv:
            pst = qkv.enter_context(tc.tile_pool(name="pst", bufs=2, space="PSUM"))
            po = qkv.enter_context(tc.tile_pool(name="po", bufs=2, space="PSUM"))
            pwork = qkv.enter_context(tc.tile_pool(name="pwork", bufs=2,
                                                   space="PSUM"))
            ptp = qkv.enter_context(tc.tile_pool(name="ptp", bufs=6))
            dnp = qkv.enter_context(tc.tile_pool(name="dn", bufs=4))
            oup = qkv.enter_context(tc.tile_pool(name="ou", bufs=6))

            filler = []
            fidx = [0]

            def emit_filler(n=1):
                while n > 0 and fidx[0] < len(filler):
                    filler[fidx[0]]()
                    fidx[0] += 1
                    n -= 1

            def attn_block(p, h, nh, emit_at=(2, 5, 8, 11, 14)):
                er = (h % 2) * 64
                o_ps = [po.tile([65, 512], F32, tag="ops",
                                name=f"ops{h}_{nh}_{i}") for i in range(2)]
                for m in range(NT):
                    ms_ = slice(m * 128, (m + 1) * 128)
                    st = pst.tile([128, 1024], F32, tag="st",
                                  name=f"st{h}_{nh}_{m}")
                    for i in range(2):
                        c = 2 * nh + i
                        cs = slice(c * 512, (c + 1) * 512)
                        nc.tensor.matmul(st[:, i * 512:(i + 1) * 512],
                                         kt[p][er:er + 64, ms_],
                                         qt[p][er:er + 64, cs],
                                         start=True, stop=True)
                    pt = ptp.tile([128, 1024], BF16, tag="pt",
                                  name=f"pt{h}_{nh}_{m}")
                    nc.scalar.activation(pt, st, AF.Exp, bias=zero_c)
                    for i in range(2):
                        nc.tensor.matmul(o_ps[i], vt_r[m][:, h, :],
                                         pt[:, i * 512:(i + 1) * 512],
                                         start=(m == 0), stop=(m == NT - 1))
                    if m in emit_at:
                        emit_filler(1)
                for i in range(2):
                    c = 2 * nh + i
                    ou = oup.tile([65, 512], F32, tag="ou", name=f"ou{h}_{c}")
                    nc.vector.tensor_copy(ou, o_ps[i])   # frees the PSUM slot
                    den_b = dnp.tile([64, 512], F32, tag="db", name=f"db{h}_{c}")
                    sr = scr[h * NCH + c:h * NCH + c + 1, :]
                    nc.gpsimd.dma_start(out=sr, in_=ou[64:65, :])
                    nc.gpsimd.dma_start(out=den_b,
                                        in_=sr.to_broadcast([64, 512]))
                    rbt = dnp.tile([64, 512], F32, tag="rb", name=f"rbt{h}_{c}")
                    nc.vector.reciprocal_approx_fast(rbt, den_b)
                    nc.vector.tensor_mul(ot[p][c][er:er + 64, :],
                                         ou[0:64, :], rbt)
                emit_filler(1)   # boundary: keep PE fed across block seams

            # -------- schedule: upfront q0/k0 + paced filler inventory.
            # Explicit static interleave: the scheduler's cost-model sim
            # does not model the HAM cold clock, so filler positions are
            # pinned in program order at a measured pace instead.
            for c in range(NCH):
                qk_group(pwork, "wk", 1, 0, c, kt)
            for c in range(NCH):
                qk_group(pwork, "wq", 0, 0, c, qt)
            for m in range(4):
                v_group(pwork, m)

            filler += [lambda m=m: v_group(pwork, m) for m in range(4, NT)]
            for p in (1, 2, 3):
                filler += [lambda c=c, p=p: qk_group(pwork, "wk", 1, p, c, kt)
                           for c in range(NCH)]
                filler += [lambda c=c, p=p: qk_group(pwork, "wq", 0, p, c, qt)
                           for c in range(NCH)]

            attn_block(0, 0, 0, emit_at=tuple(range(12)))   # V rides here
            attn_block(0, 0, 1)
            attn_block(0, 1, 0)
            attn_block(0, 1, 1)
            attn_block(1, 2, 0)
            attn_block(1, 2, 1)
            attn_block(1, 3, 0)
            attn_block(1, 3, 1)
            attn_block(2, 4, 0)
            attn_block(2, 4, 1)
            attn_block(2, 5, 0)
            attn_block(2, 5, 1)
            # pair 3 nh-major: after both nh0 blocks, out-proj chunks 0,1
            # unlock and ride the nh1 blocks.
            attn_block(3, 6, 0)
            attn_block(3, 7, 0)
            filler += [lambda tt=tt, fc=fc: outproj_group(pwork, tt, fc)
                       for tt in range(8) for fc in range(2)]
            attn_block(3, 6, 1, emit_at=(1, 3, 5, 7, 9, 11, 13))
            attn_block(3, 7, 1, emit_at=(1, 3, 5, 7, 9, 11, 13))
            emit_filler(len(filler))

        # -------- tail: out-proj chunks 2,3 (deep PSUM pipeline) --------
        with tc.tile_pool(name="ptail", bufs=4, space="PSUM") as ptail:
            for tt in range(8, NT):
                for fc in range(2):
                    outproj_group(ptail, tt, fc)


def _prep(inputs):
    x = np.asarray(inputs["x"], np.float32)
    Wq = np.asarray(inputs["Wq"], np.float32)
    Wkv = np.asarray(inputs["Wkv"], np.float32)
    Wo = np.asarray(inputs["Wo"], np.float32)
    ln_g = np.asarray(inputs["ln_g"], np.float32)
    ln_b = np.asarray(inputs["ln_b"], np.float32)
    lnc_g = np.asarray(inputs["lnc_g"], np.float32)
    lnc_b = np.asarray(inputs["lnc_b"], np.float32)

    bf = ml_dtypes.bfloat16
    qscale = np.float32(D ** -0.5)
    in_maps = []
    for c in range(8):
        b, g = c // 2, c % 2
        gs = slice(g * E, (g + 1) * E)
        Wq_g = Wq[gs] * ln_g[None, :] * qscale          # [E, F] (scale folded)
        cq = (Wq[gs] @ ln_b) * qscale                   # [E]
        Wk_g = Wkv[gs] * lnc_g[None, :]
        ck = Wkv[gs] @ lnc_b
        Wv_g = Wkv[H * D + g * E:H * D + (g + 1) * E] * lnc_g[None, :]
        cv = Wkv[H * D + g * E:H * D + (g + 1) * E] @ lnc_b
        corr = np.stack([
            np.concatenate([-Wq_g.sum(1), -Wk_g.sum(1), -Wv_g.sum(1)]),
            np.concatenate([cq, ck, cv]),
        ])                                              # [2, 3E]
        in_maps.append({
            "onesd": np.ones((128, 512), bf),
            "xT": np.ascontiguousarray(x[b].T).astype(bf),
            "wq": np.ascontiguousarray(Wq_g.T).astype(bf),
            "wk": np.ascontiguousarray(Wk_g.T).astype(bf),
            "wv": np.ascontiguousarray(Wv_g.T).astype(bf),
            "corr": np.ascontiguousarray(corr).astype(bf),
            "wo": np.ascontiguousarray(Wo[:, gs].T).astype(bf),
        })
    return in_maps


def kernel(**inputs):
    if "nc" not in _CACHE:
        _CACHE["nc"] = build_program()
    nc = _CACHE["nc"]
    in_maps = _prep(inputs)
    res = run_bass_kernel_spmd(nc, in_maps, list(range(8))).results
    x = np.asarray(inputs["x"], np.float32)
    out = np.empty((B, N, F), np.float32)
    for b in range(B):
        out[b] = (res[2 * b]["out"].astype(np.float32)
                  + res[2 * b + 1]["out"].astype(np.float32)
                  + x[b])
    return out


if __name__ == "__main__":
    import reference
    ins = {k: np.asarray(v) for k, v in reference.setup_inputs().items()}
    exp = np.asarray(reference.reference(**ins))
    got = kernel(**ins)
    err = np.abs(got - exp)
    rel = np.linalg.norm(got - exp) / np.linalg.norm(exp)
    print("max abs err:", err.max(), "rel:", rel)


# revision 16
# speedup vs baseline: 1.0228x; 1.0062x over previous
"""Trainium2 Bass kernel for nn_Attention_7962869366891.

Module: y = x + Wo @ attn(LN_q(x) Wq, LN_c(x) Wkv)   with B=4, N=2048, F=1024,
H=16 heads, D=64.

Sharding (8 cores): core c -> (batch b = c//2, head-group g = c%2 of 8 heads).
Each core computes a full [N, F] partial of the output projection for its 8
heads; the host sums the two partials per batch plus the residual skip.

Device-side design (per core), v5:
  - bf16 datapath, fp32 PSUM/stats/normalization.
  - Few big strided DMAs (per-chunk x slabs, whole-weight slabs) -- the sync
    engine was the lead-in bottleneck with per-tile DMAs.
  - Chunk-major LN stats so LN/z and the first projections start early.
  - LN affine folded into weights host-side; per-token -mu*rstd rides as one
    K=2 matmul per accumulation group.
  - Attention blocks of (head, nhalf) x 16 key-tiles: St = k^T q, exp from
    PSUM into bf16 pt, O^T = V'^T P with a ones column for the denominator.
    Block order: all nhalf=0 across pairs, then all nhalf=1, so out-proj
    chunks 0,1 unlock at the halfway point.
  - All projection / out-proj work lives at the END of program order (lowest
    priority): the Tile scheduler pulls it per-instruction exactly when the
    PE idles waiting on exp, keeping the PE dense (HAM stays warm) without
    ever preempting the attention-critical QK stream.
  - o_ps PSUM slots released fast via DVE copies; reciprocal broadcast
    (gpsimd DRAM bounce) runs off-critical-path.
"""

import numpy as np
import ml_dtypes

import concourse.bass as bass
import concourse.bacc as bacc
import concourse.mybir as mybir
import concourse.tile as tile
from concourse.bass_utils import run_bass_kernel_spmd

F32 = mybir.dt.float32
BF16 = mybir.dt.bfloat16
AF = mybir.ActivationFunctionType

B, N, F, H, D = 4, 2048, 1024, 16, 64
HG = 8                # heads per core
E = HG * D            # 512 projection dims per core
NT = N // 128         # 16 token tiles
FTC = F // 128        # 8 feature tiles
ET = E // 128         # 4 e-tiles (head pairs)
NCH = N // 512        # 4 token chunks of 512
EPS = 1e-5

_CACHE = {}


def build_program():
    nc = bacc.Bacc("TRN2", target_bir_lowering=False, debug=False, num_devices=8)

    xT = nc.dram_tensor("xT", [F, N], BF16, kind="ExternalInput").ap()
    wq = nc.dram_tensor("wq", [F, E], BF16, kind="ExternalInput").ap()
    wk = nc.dram_tensor("wk", [F, E], BF16, kind="ExternalInput").ap()
    wv = nc.dram_tensor("wv", [F, E], BF16, kind="ExternalInput").ap()
    corr = nc.dram_tensor("corr", [2, 3 * E], BF16, kind="ExternalInput").ap()
    wo = nc.dram_tensor("wo", [E, F], BF16, kind="ExternalInput").ap()
    onesd = nc.dram_tensor("onesd", [128, 512], BF16, kind="ExternalInput").ap()
    out = nc.dram_tensor("out", [N, F], BF16, kind="ExternalOutput").ap()
    scr = nc.dram_tensor("scr", [HG * NCH, 512], F32).ap()

    with tile.TileContext(nc) as tc:
        _emit(nc, tc, xT, wq, wk, wv, corr, wo, onesd, out, scr)
    nc.compile()
    return nc


def _emit(nc, tc, xT, wq, wk, wv, corr, wo, onesd, out, scr):
    from contextlib import ExitStack
    pers = ExitStack()
    with pers:
        # ---------------- persistent constants ----------------
        single = pers.enter_context(tc.tile_pool(name="single", bufs=1))
        ones128 = single.tile([128, 128], BF16)
        nc.sync.dma_start(out=ones128, in_=onesd[:, 0:128])
        zero_c = single.tile([128, 1], F32)
        nc.vector.memset(zero_c, 0.0)
        eps_c = single.tile([128, 1], F32)
        nc.vector.memset(eps_c, EPS)
        aug = [single.tile([2, 512], BF16, name=f"aug_{c}", tag=f"aug_{c}")
               for c in range(NCH)]
        for c in range(NCH):
            nc.gpsimd.dma_start(out=aug[c][1:2, :], in_=onesd[0:1, :])
        corr2 = single.tile([2, 3 * E], BF16)  # row0 = -rowsum(W'), row1 = bias
        nc.sync.dma_start(out=corr2, in_=corr)

        # ---------------- x: one big strided DMA per chunk ----------------
        xpool = pers.enter_context(tc.tile_pool(name="x", bufs=1))
        xT_r = xT.rearrange("(ft p) n -> p ft n", p=128)
        xc, xc_r = [], []
        for c in range(NCH):
            t = xpool.tile([128, FTC * 512], BF16, name=f"x_{c}", tag=f"x_{c}")
            tr = t.rearrange("p (ft n) -> p ft n", n=512)
            nc.sync.dma_start(out=tr, in_=xT_r[:, :, c * 512:(c + 1) * 512])
            xc.append(t)
            xc_r.append(tr)

        def xts(ft, c):
            return xc_r[c][:, ft, :]

        # ---------------- weights: one slab DMA each ----------------
        wpool = pers.enter_context(tc.tile_pool(name="w", bufs=1))
        wsl = {}
        for wdram, nm in ((wq, "wq"), (wk, "wk"), (wv, "wv")):
            t = wpool.tile([128, FTC * E], BF16, name=f"{nm}t", tag=f"{nm}t")
            tr = t.rearrange("p (ft e) -> p ft e", e=E)
            nc.sync.dma_start(out=tr,
                              in_=wdram.rearrange("(ft p) e -> p ft e", p=128))
            wsl[nm] = tr
        wot = wpool.tile([128, ET * F], BF16, name="wot", tag="wot")
        wot_r = wot.rearrange("p (et f) -> p et f", f=F)
        nc.sync.dma_start(out=wot_r,
                          in_=wo.rearrange("(et p) f -> p et f", p=128))

        # ---------------- LN stats (chunk-major) ----------------
        rp = pers.enter_context(tc.tile_pool(name="rp", bufs=1))
        rb = [rp.tile([128, 512], F32, name=f"rb_{c}", tag=f"rb_{c}")
              for c in range(NCH)]
        with tc.tile_pool(name="pstats", bufs=1, space="PSUM") as pstats, \
             tc.tile_pool(name="xsq", bufs=1) as xsqp, \
             tc.tile_pool(name="statf", bufs=1) as statf:
            s1l, s2l, xsl = [], [], []
            for c in range(NCH):      # all x^2 on DVE up front
                xsl.append([xsqp.tile([128, 512], BF16, tag=f"xs{c}_{ft}",
                                      name=f"xs{c}_{ft}") for ft in range(FTC)])
                for ft in range(FTC):
                    nc.vector.tensor_mul(xsl[c][ft], xts(ft, c), xts(ft, c))
            for c in range(NCH):      # s1 needs no xsq: PE starts on DMA
                s1 = pstats.tile([128, 512], F32, tag=f"s1{c}", name=f"s1_{c}")
                for ft in range(FTC):
                    nc.tensor.matmul(s1, ones128, xts(ft, c),
                                     start=(ft == 0), stop=(ft == FTC - 1))
                s1l.append(s1)
            for c in range(NCH):
                s2 = pstats.tile([128, 512], F32, tag=f"s2{c}", name=f"s2_{c}")
                for ft in range(FTC):
                    nc.tensor.matmul(s2, ones128, xsl[c][ft],
                                     start=(ft == 0), stop=(ft == FTC - 1))
                s2l.append(s2)
            mus, vas = [], []
            for c in range(NCH):
                mu = statf.tile([128, 512], F32, tag=f"mu{c}", name=f"mu_{c}")
                va = statf.tile([128, 512], F32, tag=f"va{c}", name=f"va_{c}")
                m2 = statf.tile([128, 512], F32, tag=f"m2{c}", name=f"m2_{c}")
                nc.vector.tensor_scalar_mul(mu, s1l[c], 1.0 / F)
                nc.vector.tensor_scalar_mul(va, s2l[c], 1.0 / F)
                nc.vector.tensor_mul(m2, mu, mu)
                nc.vector.tensor_sub(va, va, m2)   # var
                mus.append(mu)
                vas.append(va)
            for c in range(NCH):      # batched Ln (one table set)
                nc.scalar.activation(vas[c], vas[c], AF.Ln, bias=eps_c)
            for c in range(NCH):      # batched Exp
                nc.scalar.activation(rb[c], vas[c], AF.Exp,
                                     bias=zero_c, scale=-0.5)
            for c in range(NCH):
                nc.vector.tensor_mul(mus[c], mus[c], rb[c])    # mu*rstd
                nc.vector.tensor_copy(aug[c][0:1, :], mus[c][0:1, :])
                for ft in range(FTC):                          # z = x*rstd
                    nc.vector.tensor_mul(xts(ft, c), xts(ft, c), rb[c])

        # ---------------- result tiles ----------------
        qkpool = pers.enter_context(tc.tile_pool(name="qk", bufs=1, side="right"))
        qt = [qkpool.tile([128, N], BF16, name=f"qt_{et}", tag=f"qt_{et}")
              for et in range(ET)]
        kt = [qkpool.tile([128, N], BF16, name=f"kt_{et}", tag=f"kt_{et}")
              for et in range(ET)]
        vpool = pers.enter_context(tc.tile_pool(name="vtok", bufs=1, side="right"))
        vt = [vpool.tile([128, HG * (D + 1)], BF16, name=f"vt_{m}",
                         tag=f"vt_{m}") for m in range(NT)]
        vt_r = [t.rearrange("p (h x) -> p h x", x=D + 1) for t in vt]
        opool = pers.enter_context(tc.tile_pool(name="ostk", bufs=1, side="right"))
        ot = [[opool.tile([128, 512], BF16, name=f"ot_{et}_{c}",
                          tag=f"ot_{et}_{c}") for c in range(NCH)]
              for et in range(ET)]
        obp = pers.enter_context(tc.tile_pool(name="obp", bufs=4))

        def qk_group(pool, wnm, wi, et, c, dest):
            crA = corr2[:, wi * E + et * 128: wi * E + (et + 1) * 128]
            ps = pool.tile([128, 512], F32, tag="pp", name=f"pp{wi}_{et}_{c}")
            for ft in range(FTC):
                nc.tensor.matmul(ps, wsl[wnm][:, ft, et * 128:(et + 1) * 128],
                                 xts(ft, c), start=(ft == 0), stop=False)
            nc.tensor.matmul(ps, crA, aug[c], start=False, stop=True)
            nc.vector.tensor_copy(dest[et][:, c * 512:(c + 1) * 512], ps)

        def v_group(pool, m):
            c, js = m // 4, slice((m % 4) * 128, (m % 4 + 1) * 128)
            nc.gpsimd.dma_start(out=vt_r[m][:, :, D:D + 1], in_=onesd[:, 0:HG])
            ps = pool.tile([128, 512], F32, tag="pp", name=f"ppv_{m}")
            for ft in range(FTC):
                nc.tensor.matmul(ps, xts(ft, c)[:, js], wsl["wv"][:, ft, :],
                                 start=(ft == 0), stop=False)
            nc.tensor.matmul(ps, aug[c][:, js], corr2[:, 2 * E:3 * E],
                             start=False, stop=True)
            nc.vector.tensor_copy(vt_r[m][:, :, 0:D],
                                  ps.rearrange("p (h d) -> p h d", d=D))

        def outproj_group(pool, tt, fc):
            c = tt // 4
            js = slice((tt % 4) * 128, (tt % 4 + 1) * 128)
            ts_ = slice(tt * 128, (tt + 1) * 128)
            fs = slice(fc * 512, (fc + 1) * 512)
            ps = pool.tile([128, 512], F32, tag="pp", name=f"pso{tt}_{fc}")
            for et in range(ET):
                nc.tensor.matmul(ps, ot[et][c][:, js], wot_r[:, et, fs],
                                 start=(et == 0), stop=(et == ET - 1))
            ob = obp.tile([128, 512], BF16, tag="ob", name=f"ob{tt}_{fc}")
            nc.vector.tensor_copy(ob, ps)
            nc.sync.dma_start(out=out[ts_, fs], in_=ob)

        qkv = ExitStack()
        with qkv:
            pst = qkv.enter_context(tc.tile_pool(name="pst", bufs=2, space="PSUM"))
            po = qkv.enter_context(tc.tile_pool(name="po", bufs=2, space="PSUM"))
            pwork = qkv.enter_context(tc.tile_pool(name="pwork", bufs=2,
                                                   space="PSUM"))
            ptp = qkv.enter_context(tc.tile_pool(name="ptp", bufs=6))
            dnp = qkv.enter_context(tc.tile_pool(name="dn", bufs=4))
            oup = qkv.enter_context(tc.tile_pool(name="ou", bufs=6))

            filler = []
            fidx = [0]

            def emit_filler(n=1):
                while n > 0 and fidx[0] < len(filler):
                    filler[fidx[0]]()
                    fidx[0] += 1
                    n -= 1

            def attn_block(p, h, nh, emit_at=(3, 8, 13)):
                er = (h % 2) * 64
                o_ps = [po.tile([65, 512], F32, tag="ops",
                                name=f"ops{h}_{nh}_{i}") for i in range(2)]
                for m in range(NT):
                    ms_ = slice(m * 128, (m + 1) * 128)
                    st = pst.tile([128, 1024], F32, tag="st",
                                  name=f"st{h}_{nh}_{m}")
                    for i in range(2):
                        c = 2 * nh + i
                        cs = slice(c * 512, (c + 1) * 512)
                        nc.tensor.matmul(st[:, i * 512:(i + 1) * 512],
                                         kt[p][er:er + 64, ms_],
                                         qt[p][er:er + 64, cs],
                                         start=True, stop=True)
                    pt = ptp.tile([128, 1024], BF16, tag="pt",
                                  name=f"pt{h}_{nh}_{m}")
                    nc.scalar.activation(pt, st, AF.Exp, bias=zero_c)
                    for i in range(2):
                        nc.tensor.matmul(o_ps[i], vt_r[m][:, h, :],
                                         pt[:, i * 512:(i + 1) * 512],
                                         start=(m == 0), stop=(m == NT - 1))
                    if m in emit_at:
                        emit_filler(1)
                for i in range(2):
                    c = 2 * nh + i
                    ou = oup.tile([65, 512], F32, tag="ou", name=f"ou{h}_{c}")
                    nc.vector.tensor_copy(ou, o_ps[i])   # frees the PSUM slot
                    den_b = dnp.tile([64, 512], F32, tag="db", name=f"db{h}_{c}")
                    sr = scr[h * NCH + c:h * NCH + c + 1, :]
                    nc.gpsimd.dma_start(out=sr, in_=ou[64:65, :])
                    nc.gpsimd.dma_start(out=den_b,
                                        in_=sr.to_broadcast([64, 512]))
                    rbt = dnp.tile([64, 512], F32, tag="rb", name=f"rbt{h}_{c}")
                    nc.vector.reciprocal_approx_fast(rbt, den_b)
                    nc.vector.tensor_mul(ot[p][c][er:er + 64, :],
                                         ou[0:64, :], rbt)
                emit_filler(1)   # boundary: keep PE fed across block seams

            # -------- schedule: upfront q0/k0 + paced filler inventory.
            # Explicit static interleave: the scheduler's cost-model sim
            # does not model the HAM cold clock, so filler positions are
            # pinned in program order at a measured pace instead.
            for c in range(NCH):
                qk_group(pwork, "wk", 1, 0, c, kt)
            for c in range(NCH):
                qk_group(pwork, "wq", 0, 0, c, qt)
            for m in range(4):
                v_group(pwork, m)

            filler += [lambda m=m: v_group(pwork, m) for m in range(4, NT)]
            for p in (1, 2, 3):
                filler += [lambda c=c, p=p: qk_group(pwork, "wk", 1, p, c, kt)
                           for c in range(NCH)]
                filler += [lambda c=c, p=p: qk_group(pwork, "wq", 0, p, c, qt)
                           for c in range(NCH)]

            attn_block(0, 0, 0, emit_at=tuple(range(12)))   # V rides here
            attn_block(0, 0, 1)
            attn_block(0, 1, 0)
            attn_block(0, 1, 1)
            attn_block(1, 2, 0)
            attn_block(1, 2, 1)
            attn_block(1, 3, 0)
            attn_block(1, 3, 1)
            attn_block(2, 4, 0)
            attn_block(2, 4, 1)
            attn_block(2, 5, 0)
            attn_block(2, 5, 1)
            # pair 3 nh-major: after both nh0 blocks, out-proj chunks 0,1
            # unlock and ride the nh1 blocks.
            attn_block(3, 6, 0)
            attn_block(3, 7, 0)
            filler += [lambda tt=tt, fc=fc: outproj_group(pwork, tt, fc)
                       for tt in range(8) for fc in range(2)]
            attn_block(3, 6, 1, emit_at=(1, 3, 5, 7, 9, 11, 13))
            attn_block(3, 7, 1, emit_at=(1, 3, 5, 7, 9, 11, 13))
            emit_filler(len(filler))

        # -------- tail: out-proj chunks 2,3 (deep PSUM pipeline) --------
        with tc.tile_pool(name="ptail", bufs=4, space="PSUM") as ptail:
            for tt in range(8, NT):
                for fc in range(2):
                    outproj_group(ptail, tt, fc)


def _prep(inputs):
    x = np.asarray(inputs["x"], np.float32)
    Wq = np.asarray(inputs["Wq"], np.float32)
    Wkv = np.asarray(inputs["Wkv"], np.float32)
    Wo = np.asarray(inputs["Wo"], np.float32)
    ln_g = np.asarray(inputs["ln_g"], np.float32)
    ln_b = np.asarray(inputs["ln_b"], np.float32)
    lnc_g = np.asarray(inputs["lnc_g"], np.float32)
    lnc_b = np.asarray(inputs["lnc_b"], np.float32)

    bf = ml_dtypes.bfloat16
    qscale = np.float32(D ** -0.5)
    in_maps = []
    for c in range(8):
        b, g = c // 2, c % 2
        gs = slice(g * E, (g + 1) * E)
        Wq_g = Wq[gs] * ln_g[None, :] * qscale          # [E, F] (scale folded)
        cq = (Wq[gs] @ ln_b) * qscale                   # [E]
        Wk_g = Wkv[gs] * lnc_g[None, :]
        ck = Wkv[gs] @ lnc_b
        Wv_g = Wkv[H * D + g * E:H * D + (g + 1) * E] * lnc_g[None, :]
        cv = Wkv[H * D + g * E:H * D + (g + 1) * E] @ lnc_b
        corr = np.stack([
            np.concatenate([-Wq_g.sum(1), -Wk_g.sum(1), -Wv_g.sum(1)]),
            np.concatenate([cq, ck, cv]),
        ])                                              # [2, 3E]
        in_maps.append({
            "onesd": np.ones((128, 512), bf),
            "xT": np.ascontiguousarray(x[b].T).astype(bf),
            "wq": np.ascontiguousarray(Wq_g.T).astype(bf),
            "wk": np.ascontiguousarray(Wk_g.T).astype(bf),
            "wv": np.ascontiguousarray(Wv_g.T).astype(bf),
            "corr": np.ascontiguousarray(corr).astype(bf),
            "wo": np.ascontiguousarray(Wo[:, gs].T).astype(bf),
        })
    return in_maps


def kernel(**inputs):
    if "nc" not in _CACHE:
        _CACHE["nc"] = build_program()
    nc = _CACHE["nc"]
    in_maps = _prep(inputs)
    res = run_bass_kernel_spmd(nc, in_maps, list(range(8))).results
    x = np.asarray(inputs["x"], np.float32)
    out = np.empty((B, N, F), np.float32)
    for b in range(B):
        out[b] = (res[2 * b]["out"].astype(np.float32)
                  + res[2 * b + 1]["out"].astype(np.float32)
                  + x[b])
    return out


if __name__ == "__main__":
    import reference
    ins = {k: np.asarray(v) for k, v in reference.setup_inputs().items()}
    exp = np.asarray(reference.reference(**ins))
    got = kernel(**ins)
    err = np.abs(got - exp)
    rel = np.linalg.norm(got - exp) / np.linalg.norm(exp)
    print("max abs err:", err.max(), "rel:", rel)


# revision 17
# speedup vs baseline: 1.0300x; 1.0071x over previous
"""Trainium2 Bass kernel for nn_Attention_7962869366891.

Module: y = x + Wo @ attn(LN_q(x) Wq, LN_c(x) Wkv)   with B=4, N=2048, F=1024,
H=16 heads, D=64.

Sharding (8 cores): core c -> (batch b = c//2, head-group g = c%2 of 8 heads).
Each core computes a full [N, F] partial of the output projection for its 8
heads; the host sums the two partials per batch plus the residual skip.

Device-side design (per core), v5:
  - bf16 datapath, fp32 PSUM/stats/normalization.
  - Few big strided DMAs (per-chunk x slabs, whole-weight slabs) -- the sync
    engine was the lead-in bottleneck with per-tile DMAs.
  - Chunk-major LN stats so LN/z and the first projections start early.
  - LN affine folded into weights host-side; per-token -mu*rstd rides as one
    K=2 matmul per accumulation group.
  - Attention blocks of (head, nhalf) x 16 key-tiles: St = k^T q, exp from
    PSUM into bf16 pt, O^T = V'^T P with a ones column for the denominator.
    Block order: all nhalf=0 across pairs, then all nhalf=1, so out-proj
    chunks 0,1 unlock at the halfway point.
  - All projection / out-proj work lives at the END of program order (lowest
    priority): the Tile scheduler pulls it per-instruction exactly when the
    PE idles waiting on exp, keeping the PE dense (HAM stays warm) without
    ever preempting the attention-critical QK stream.
  - o_ps PSUM slots released fast via DVE copies; reciprocal broadcast
    (gpsimd DRAM bounce) runs off-critical-path.
"""

import numpy as np
import ml_dtypes

import concourse.bass as bass
import concourse.bacc as bacc
import concourse.mybir as mybir
import concourse.tile as tile
from concourse.bass_utils import run_bass_kernel_spmd

F32 = mybir.dt.float32
BF16 = mybir.dt.bfloat16
AF = mybir.ActivationFunctionType

B, N, F, H, D = 4, 2048, 1024, 16, 64
HG = 8                # heads per core
E = HG * D            # 512 projection dims per core
NT = N // 128         # 16 token tiles
FTC = F // 128        # 8 feature tiles
ET = E // 128         # 4 e-tiles (head pairs)
NCH = N // 512        # 4 token chunks of 512
EPS = 1e-5

_CACHE = {}


def build_program():
    nc = bacc.Bacc("TRN2", target_bir_lowering=False, debug=False, num_devices=8)

    xT = nc.dram_tensor("xT", [F, N], BF16, kind="ExternalInput").ap()
    wq = nc.dram_tensor("wq", [F, E], BF16, kind="ExternalInput").ap()
    wk = nc.dram_tensor("wk", [F, E], BF16, kind="ExternalInput").ap()
    wv = nc.dram_tensor("wv", [F, E], BF16, kind="ExternalInput").ap()
    corr = nc.dram_tensor("corr", [2, 3 * E], BF16, kind="ExternalInput").ap()
    wo = nc.dram_tensor("wo", [E, F], BF16, kind="ExternalInput").ap()
    onesd = nc.dram_tensor("onesd", [128, 512], BF16, kind="ExternalInput").ap()
    out = nc.dram_tensor("out", [N, F], BF16, kind="ExternalOutput").ap()
    scr = nc.dram_tensor("scr", [HG * NCH, 512], F32).ap()

    with tile.TileContext(nc) as tc:
        _emit(nc, tc, xT, wq, wk, wv, corr, wo, onesd, out, scr)
    nc.compile()
    return nc


def _emit(nc, tc, xT, wq, wk, wv, corr, wo, onesd, out, scr):
    from contextlib import ExitStack
    pers = ExitStack()
    with pers:
        # ---------------- persistent constants ----------------
        single = pers.enter_context(tc.tile_pool(name="single", bufs=1))
        ones128 = single.tile([128, 128], BF16)
        nc.sync.dma_start(out=ones128, in_=onesd[:, 0:128])
        zero_c = single.tile([128, 1], F32)
        nc.vector.memset(zero_c, 0.0)
        eps_c = single.tile([128, 1], F32)
        nc.vector.memset(eps_c, EPS)
        aug = [single.tile([2, 512], BF16, name=f"aug_{c}", tag=f"aug_{c}")
               for c in range(NCH)]
        for c in range(NCH):
            nc.gpsimd.dma_start(out=aug[c][1:2, :], in_=onesd[0:1, :])
        corr2 = single.tile([2, 3 * E], BF16)  # row0 = -rowsum(W'), row1 = bias
        nc.sync.dma_start(out=corr2, in_=corr)

        # ---------------- x: one big strided DMA per chunk ----------------
        xpool = pers.enter_context(tc.tile_pool(name="x", bufs=1))
        xT_r = xT.rearrange("(ft p) n -> p ft n", p=128)
        xc, xc_r = [], []
        for c in range(NCH):
            t = xpool.tile([128, FTC * 512], BF16, name=f"x_{c}", tag=f"x_{c}")
            tr = t.rearrange("p (ft n) -> p ft n", n=512)
            nc.sync.dma_start(out=tr, in_=xT_r[:, :, c * 512:(c + 1) * 512])
            xc.append(t)
            xc_r.append(tr)

        def xts(ft, c):
            return xc_r[c][:, ft, :]

        # ---------------- weights: one slab DMA each ----------------
        wpool = pers.enter_context(tc.tile_pool(name="w", bufs=1))
        wsl = {}
        for wdram, nm in ((wq, "wq"), (wk, "wk"), (wv, "wv")):
            t = wpool.tile([128, FTC * E], BF16, name=f"{nm}t", tag=f"{nm}t")
            tr = t.rearrange("p (ft e) -> p ft e", e=E)
            nc.sync.dma_start(out=tr,
                              in_=wdram.rearrange("(ft p) e -> p ft e", p=128))
            wsl[nm] = tr
        wot = wpool.tile([128, ET * F], BF16, name="wot", tag="wot")
        wot_r = wot.rearrange("p (et f) -> p et f", f=F)
        nc.sync.dma_start(out=wot_r,
                          in_=wo.rearrange("(et p) f -> p et f", p=128))

        # ---------------- LN stats (chunk-major) ----------------
        rp = pers.enter_context(tc.tile_pool(name="rp", bufs=1))
        rb = [rp.tile([128, 512], F32, name=f"rb_{c}", tag=f"rb_{c}")
              for c in range(NCH)]
        with tc.tile_pool(name="pstats", bufs=1, space="PSUM") as pstats, \
             tc.tile_pool(name="xsq", bufs=1) as xsqp, \
             tc.tile_pool(name="statf", bufs=1) as statf:
            s1l, s2l, xsl = [], [], []
            for c in range(NCH):      # all x^2 on DVE up front
                xsl.append([xsqp.tile([128, 512], BF16, tag=f"xs{c}_{ft}",
                                      name=f"xs{c}_{ft}") for ft in range(FTC)])
                for ft in range(FTC):
                    nc.vector.tensor_mul(xsl[c][ft], xts(ft, c), xts(ft, c))
            for c in range(NCH):      # s1 needs no xsq: PE starts on DMA
                s1 = pstats.tile([128, 512], F32, tag=f"s1{c}", name=f"s1_{c}")
                for ft in range(FTC):
                    nc.tensor.matmul(s1, ones128, xts(ft, c),
                                     start=(ft == 0), stop=(ft == FTC - 1))
                s1l.append(s1)
            for c in range(NCH):
                s2 = pstats.tile([128, 512], F32, tag=f"s2{c}", name=f"s2_{c}")
                for ft in range(FTC):
                    nc.tensor.matmul(s2, ones128, xsl[c][ft],
                                     start=(ft == 0), stop=(ft == FTC - 1))
                s2l.append(s2)
            mus, vas = [], []
            for c in range(NCH):
                mu = statf.tile([128, 512], F32, tag=f"mu{c}", name=f"mu_{c}")
                va = statf.tile([128, 512], F32, tag=f"va{c}", name=f"va_{c}")
                m2 = statf.tile([128, 512], F32, tag=f"m2{c}", name=f"m2_{c}")
                nc.vector.tensor_scalar_mul(mu, s1l[c], 1.0 / F)
                nc.vector.tensor_scalar_mul(va, s2l[c], 1.0 / F)
                nc.vector.tensor_mul(m2, mu, mu)
                nc.vector.tensor_sub(va, va, m2)   # var
                mus.append(mu)
                vas.append(va)
            for c in range(NCH):      # batched Ln (one table set)
                nc.scalar.activation(vas[c], vas[c], AF.Ln, bias=eps_c)
            for c in range(NCH):      # batched Exp
                nc.scalar.activation(rb[c], vas[c], AF.Exp,
                                     bias=zero_c, scale=-0.5)
            for c in range(NCH):
                nc.vector.tensor_mul(mus[c], mus[c], rb[c])    # mu*rstd
                nc.vector.tensor_copy(aug[c][0:1, :], mus[c][0:1, :])
                for ft in range(FTC):                          # z = x*rstd
                    nc.vector.tensor_mul(xts(ft, c), xts(ft, c), rb[c])

        # ---------------- result tiles ----------------
        qkpool = pers.enter_context(tc.tile_pool(name="qk", bufs=1, side="right"))
        qt = [qkpool.tile([128, N], BF16, name=f"qt_{et}", tag=f"qt_{et}")
              for et in range(ET)]
        kt = [qkpool.tile([128, N], BF16, name=f"kt_{et}", tag=f"kt_{et}")
              for et in range(ET)]
        vpool = pers.enter_context(tc.tile_pool(name="vtok", bufs=1, side="right"))
        vt = [vpool.tile([128, HG * (D + 1)], BF16, name=f"vt_{m}",
                         tag=f"vt_{m}") for m in range(NT)]
        vt_r = [t.rearrange("p (h x) -> p h x", x=D + 1) for t in vt]
        opool = pers.enter_context(tc.tile_pool(name="ostk", bufs=1, side="right"))
        ot = [[opool.tile([128, 512], BF16, name=f"ot_{et}_{c}",
                          tag=f"ot_{et}_{c}") for c in range(NCH)]
              for et in range(ET)]
        obp = pers.enter_context(tc.tile_pool(name="obp", bufs=4))

        def qk_group(pool, wnm, wi, et, c, dest):
            crA = corr2[:, wi * E + et * 128: wi * E + (et + 1) * 128]
            ps = pool.tile([128, 512], F32, tag="pp", name=f"pp{wi}_{et}_{c}")
            for ft in range(FTC):
                nc.tensor.matmul(ps, wsl[wnm][:, ft, et * 128:(et + 1) * 128],
                                 xts(ft, c), start=(ft == 0), stop=False)
            nc.tensor.matmul(ps, crA, aug[c], start=False, stop=True)
            nc.vector.tensor_copy(dest[et][:, c * 512:(c + 1) * 512], ps)

        def v_group(pool, m):
            c, js = m // 4, slice((m % 4) * 128, (m % 4 + 1) * 128)
            nc.gpsimd.dma_start(out=vt_r[m][:, :, D:D + 1], in_=onesd[:, 0:HG])
            ps = pool.tile([128, 512], F32, tag="pp", name=f"ppv_{m}")
            for ft in range(FTC):
                nc.tensor.matmul(ps, xts(ft, c)[:, js], wsl["wv"][:, ft, :],
                                 start=(ft == 0), stop=False)
            nc.tensor.matmul(ps, aug[c][:, js], corr2[:, 2 * E:3 * E],
                             start=False, stop=True)
            nc.vector.tensor_copy(vt_r[m][:, :, 0:D],
                                  ps.rearrange("p (h d) -> p h d", d=D))

        def outproj_group(pool, tt, fc):
            c = tt // 4
            js = slice((tt % 4) * 128, (tt % 4 + 1) * 128)
            ts_ = slice(tt * 128, (tt + 1) * 128)
            fs = slice(fc * 512, (fc + 1) * 512)
            ps = pool.tile([128, 512], F32, tag="pp", name=f"pso{tt}_{fc}")
            for et in range(ET):
                nc.tensor.matmul(ps, ot[et][c][:, js], wot_r[:, et, fs],
                                 start=(et == 0), stop=(et == ET - 1))
            ob = obp.tile([128, 512], BF16, tag="ob", name=f"ob{tt}_{fc}")
            nc.vector.tensor_copy(ob, ps)
            nc.sync.dma_start(out=out[ts_, fs], in_=ob)

        qkv = ExitStack()
        with qkv:
            pst = qkv.enter_context(tc.tile_pool(name="pst", bufs=2, space="PSUM"))
            po = qkv.enter_context(tc.tile_pool(name="po", bufs=2, space="PSUM"))
            pwork = qkv.enter_context(tc.tile_pool(name="pwork", bufs=2,
                                                   space="PSUM"))
            ptp = qkv.enter_context(tc.tile_pool(name="ptp", bufs=6))
            dnp = qkv.enter_context(tc.tile_pool(name="dn", bufs=4))
            oup = qkv.enter_context(tc.tile_pool(name="ou", bufs=6))

            filler = []
            fidx = [0]

            def emit_filler(n=1):
                while n > 0 and fidx[0] < len(filler):
                    filler[fidx[0]]()
                    fidx[0] += 1
                    n -= 1

            def attn_block(p, h, nh, emit_at=(5, 12)):
                er = (h % 2) * 64
                o_ps = [po.tile([65, 512], F32, tag="ops",
                                name=f"ops{h}_{nh}_{i}") for i in range(2)]
                for m in range(NT):
                    ms_ = slice(m * 128, (m + 1) * 128)
                    st = pst.tile([128, 1024], F32, tag="st",
                                  name=f"st{h}_{nh}_{m}")
                    for i in range(2):
                        c = 2 * nh + i
                        cs = slice(c * 512, (c + 1) * 512)
                        nc.tensor.matmul(st[:, i * 512:(i + 1) * 512],
                                         kt[p][er:er + 64, ms_],
                                         qt[p][er:er + 64, cs],
                                         start=True, stop=True)
                    pt = ptp.tile([128, 1024], BF16, tag="pt",
                                  name=f"pt{h}_{nh}_{m}")
                    nc.scalar.activation(pt, st, AF.Exp, bias=zero_c)
                    for i in range(2):
                        nc.tensor.matmul(o_ps[i], vt_r[m][:, h, :],
                                         pt[:, i * 512:(i + 1) * 512],
                                         start=(m == 0), stop=(m == NT - 1))
                    if m in emit_at:
                        emit_filler(1)
                for i in range(2):
                    c = 2 * nh + i
                    ou = oup.tile([65, 512], F32, tag="ou", name=f"ou{h}_{c}")
                    nc.vector.tensor_copy(ou, o_ps[i])   # frees the PSUM slot
                    den_b = dnp.tile([64, 512], F32, tag="db", name=f"db{h}_{c}")
                    sr = scr[h * NCH + c:h * NCH + c + 1, :]
                    nc.gpsimd.dma_start(out=sr, in_=ou[64:65, :])
                    nc.gpsimd.dma_start(out=den_b,
                                        in_=sr.to_broadcast([64, 512]))
                    rbt = dnp.tile([64, 512], F32, tag="rb", name=f"rbt{h}_{c}")
                    nc.vector.reciprocal_approx_fast(rbt, den_b)
                    nc.vector.tensor_mul(ot[p][c][er:er + 64, :],
                                         ou[0:64, :], rbt)
                emit_filler(1)   # boundary: keep PE fed across block seams

            # -------- schedule: upfront q0/k0 + paced filler inventory.
            # Explicit static interleave: the scheduler's cost-model sim
            # does not model the HAM cold clock, so filler positions are
            # pinned in program order at a measured pace instead.
            for c in range(NCH):
                qk_group(pwork, "wk", 1, 0, c, kt)
            for c in range(NCH):
                qk_group(pwork, "wq", 0, 0, c, qt)
            for m in range(4):
                v_group(pwork, m)

            filler += [lambda m=m: v_group(pwork, m) for m in range(4, NT)]
            for p in (1, 2, 3):
                filler += [lambda c=c, p=p: qk_group(pwork, "wk", 1, p, c, kt)
                           for c in range(NCH)]
                filler += [lambda c=c, p=p: qk_group(pwork, "wq", 0, p, c, qt)
                           for c in range(NCH)]

            attn_block(0, 0, 0, emit_at=tuple(range(12)))   # V rides here
            attn_block(0, 0, 1)
            attn_block(0, 1, 0)
            attn_block(0, 1, 1)
            attn_block(1, 2, 0)
            attn_block(1, 2, 1)
            attn_block(1, 3, 0)
            attn_block(1, 3, 1)
            attn_block(2, 4, 0)
            attn_block(2, 4, 1)
            attn_block(2, 5, 0)
            attn_block(2, 5, 1)
            # pair 3 nh-major: after both nh0 blocks, out-proj chunks 0,1
            # unlock and ride the nh1 blocks.
            attn_block(3, 6, 0)
            attn_block(3, 7, 0)
            filler += [lambda tt=tt, fc=fc: outproj_group(pwork, tt, fc)
                       for tt in range(8) for fc in range(2)]
            attn_block(3, 6, 1, emit_at=(1, 3, 5, 7, 9, 11, 13))
            attn_block(3, 7, 1, emit_at=(1, 3, 5, 7, 9, 11, 13))
            emit_filler(len(filler))

        # -------- tail: out-proj chunks 2,3 (deep PSUM pipeline) --------
        with tc.tile_pool(name="ptail", bufs=4, space="PSUM") as ptail:
            for tt in range(8, NT):
                for fc in range(2):
                    outproj_group(ptail, tt, fc)


def _prep(inputs):
    x = np.asarray(inputs["x"], np.float32)
    Wq = np.asarray(inputs["Wq"], np.float32)
    Wkv = np.asarray(inputs["Wkv"], np.float32)
    Wo = np.asarray(inputs["Wo"], np.float32)
    ln_g = np.asarray(inputs["ln_g"], np.float32)
    ln_b = np.asarray(inputs["ln_b"], np.float32)
    lnc_g = np.asarray(inputs["lnc_g"], np.float32)
    lnc_b = np.asarray(inputs["lnc_b"], np.float32)

    bf = ml_dtypes.bfloat16
    qscale = np.float32(D ** -0.5)
    in_maps = []
    for c in range(8):
        b, g = c // 2, c % 2
        gs = slice(g * E, (g + 1) * E)
        Wq_g = Wq[gs] * ln_g[None, :] * qscale          # [E, F] (scale folded)
        cq = (Wq[gs] @ ln_b) * qscale                   # [E]
        Wk_g = Wkv[gs] * lnc_g[None, :]
        ck = Wkv[gs] @ lnc_b
        Wv_g = Wkv[H * D + g * E:H * D + (g + 1) * E] * lnc_g[None, :]
        cv = Wkv[H * D + g * E:H * D + (g + 1) * E] @ lnc_b
        corr = np.stack([
            np.concatenate([-Wq_g.sum(1), -Wk_g.sum(1), -Wv_g.sum(1)]),
            np.concatenate([cq, ck, cv]),
        ])                                              # [2, 3E]
        in_maps.append({
            "onesd": np.ones((128, 512), bf),
            "xT": np.ascontiguousarray(x[b].T).astype(bf),
            "wq": np.ascontiguousarray(Wq_g.T).astype(bf),
            "wk": np.ascontiguousarray(Wk_g.T).astype(bf),
            "wv": np.ascontiguousarray(Wv_g.T).astype(bf),
            "corr": np.ascontiguousarray(corr).astype(bf),
            "wo": np.ascontiguousarray(Wo[:, gs].T).astype(bf),
        })
    return in_maps


def kernel(**inputs):
    if "nc" not in _CACHE:
        _CACHE["nc"] = build_program()
    nc = _CACHE["nc"]
    in_maps = _prep(inputs)
    res = run_bass_kernel_spmd(nc, in_maps, list(range(8))).results
    x = np.asarray(inputs["x"], np.float32)
    out = np.empty((B, N, F), np.float32)
    for b in range(B):
        out[b] = (res[2 * b]["out"].astype(np.float32)
                  + res[2 * b + 1]["out"].astype(np.float32)
                  + x[b])
    return out


if __name__ == "__main__":
    import reference
    ins = {k: np.asarray(v) for k, v in reference.setup_inputs().items()}
    exp = np.asarray(reference.reference(**ins))
    got = kernel(**ins)
    err = np.abs(got - exp)
    rel = np.linalg.norm(got - exp) / np.linalg.norm(exp)
    print("max abs err:", err.max(), "rel:", rel)
